# revision 1
# baseline (speedup 1.0000x reference)
"""Trainium2 Bass kernel for nn_Attn_30820685316537 (segment_reduce attention).

Reference computation (per batch b):
    score = output @ context^T                     [Q, S]
    avg   = per-segment mean of score over S, broadcast back
    align = softmax(avg, axis=S)                   [Q, S]
    ac    = align @ context                        [Q, D]
    out   = tanh(concat(ac, output) @ W^T + bias)  [Q, D]
    returns (out, align)

Key algebraic structure exploited on device: `avg` is constant within each of
the 64 contiguous segments, so the whole pipeline factors through rank-64
segment space.  With Csum[n, d] = sum_{s in segment n} context[s, d]:
    segavg[q, n]  = (output[q, :] . Csum[n, :]) / max(cnt[n], 1)
    Enorm[q, n]   = softmax weights per segment (exp/sum with exact counts)
    align[q, s]   = Enorm[q, seg[s]]        (broadcast via 0/1 one-hot matmul)
    ac[q, d]      = sum_n Enorm[q, n] * Csum[n, d]
This removes both S-sized GEMMs while computing the exact same function
(bilinearity of the segment sum; fp reassociation only).

Sharding: data-parallel over batch B=16 across 8 NeuronCores (2 batches per
core); W replicated.  Matmuls run as float32r (full-rate fp32 mode of the PE,
~1e-4 class rel err); the softmax middle section stays in exact fp32.

Emission order software-pipelines the two batches (front(0), front(1),
back(0), back(1)) so the PE always has independent work during each batch's
serial softmax chain; input DMAs ride the Sync HWDGE ring, output DMAs the
Scalar ring to avoid head-of-line blocking.
"""
import numpy as np
from contextlib import ExitStack

B, Q, S, D = 16, 512, 1024, 1024
NSEG = 64
NCORES = 8
BPC = B // NCORES          # batches per core
QT = Q // 128              # 4 q-tiles
ST = S // 128              # 8 s-tiles
DT = D // 128              # 8 d-tiles
FT = 2 * D // 128          # 16 f-tiles of W^T

_CACHE = {}


def _build_nc():
    import concourse.bacc as bacc
    import concourse.tile as tile
    import concourse.mybir as mybir

    f32 = mybir.dt.float32
    f32r = mybir.dt.float32r
    f16 = mybir.dt.float16

    nc = bacc.Bacc("TRN2", target_bir_lowering=False, debug=False,
                   enable_asserts=False, num_devices=NCORES)

    ot_in = nc.dram_tensor("ot_in", [BPC, D, Q], f32r, kind="ExternalInput")  # output^T
    context_in = nc.dram_tensor("context_in", [BPC, S, D], f32r, kind="ExternalInput")
    wt_in = nc.dram_tensor("wt_in", [2, 2 * D, D // 2], f16, kind="ExternalInput")  # W^T e-halves, fp16
    biasr_in = nc.dram_tensor("biasr_in", [1, D], f16, kind="ExternalInput")    # bias row fp16
    ones_in = nc.dram_tensor("ones_in", [1, 128], f16, kind="ExternalInput")
    ident_in = nc.dram_tensor("ident_in", [128, 128], f32, kind="ExternalInput")
    identr_in = nc.dram_tensor("identr_in", [128, 128], f32r, kind="ExternalInput")
    oh_in = nc.dram_tensor("oh_in", [BPC, 128, ST * NSEG], f32r, kind="ExternalInput")
    invc_in = nc.dram_tensor("invc_in", [BPC, NSEG, 1], f32, kind="ExternalInput")
    cntb_in = nc.dram_tensor("cntb_in", [BPC, 128, QT * NSEG], f32, kind="ExternalInput")

    out_o = nc.dram_tensor("out_o", [BPC, Q, D], f32, kind="ExternalOutput")
    align_o = nc.dram_tensor("align_o", [BPC, Q, S], f32, kind="ExternalOutput")

    Exp = mybir.ActivationFunctionType.Exp
    Tanh = mybir.ActivationFunctionType.Tanh

    with tile.TileContext(nc) as tc, ExitStack() as ctx:
        consts = ctx.enter_context(tc.tile_pool(name="consts", bufs=1))
        wt_pool = ctx.enter_context(tc.tile_pool(name="wt", bufs=1))
        aux = ctx.enter_context(tc.tile_pool(name="aux", bufs=2))
        c_pool = ctx.enter_context(tc.tile_pool(name="cp", bufs=7))
        ot_pool = ctx.enter_context(tc.tile_pool(name="otp", bufs=2))
        act_pool = ctx.enter_context(tc.tile_pool(name="actp", bufs=2))
        mid = ctx.enter_context(tc.tile_pool(name="mid", bufs=2))      # live into back()
        mid1 = ctx.enter_context(tc.tile_pool(name="mid1", bufs=1))    # front-transient
        stage = ctx.enter_context(tc.tile_pool(name="stage", bufs=2))

        ps_cs = ctx.enter_context(tc.tile_pool(name="ps_cs", bufs=2, space="PSUM"))
        ps_sm = ctx.enter_context(tc.tile_pool(name="ps_sm", bufs=2, space="PSUM"))
        ps_mm = ctx.enter_context(tc.tile_pool(name="ps_mm", bufs=4, space="PSUM"))

        ident = consts.tile([128, 128], f32, tag="ident")
        nc.sync.dma_start(ident[:], ident_in.ap())
        identr = consts.tile([128, 128], f32r, tag="identr")
        nc.sync.dma_start(identr[:], identr_in.ap())
        biasr_sb = consts.tile([1, D], f16, tag="biasr")
        ones_sb = consts.tile([1, 128], f16, tag="ones")
        wt_sb = []
        state = [dict() for _ in range(BPC)]

        def emit_loads_aux(b, eng):
            st = state[b]
            oh = aux.tile([128, ST * NSEG], f32r, tag="oh")
            eng.dma_start(oh[:], oh_in.ap()[b])
            invc = aux.tile([NSEG, 1], f32, tag="invc")
            eng.dma_start(invc[:], invc_in.ap()[b])
            cntb = aux.tile([128, QT * NSEG], f32, tag="cntb")
            eng.dma_start(cntb[:], cntb_in.ap()[b])
            st["oh"], st["invc"], st["cntb"] = oh, invc, cntb

        def emit_loads_c(b, eng):
            st = state[b]
            c_sb = []
            for i in range(ST):
                c_i = c_pool.tile([128, D], f32r, tag="c")
                eng.dma_start(c_i[:], context_in.ap()[b, 128 * i:128 * (i + 1), :])
                c_sb.append(c_i)
            st["c"] = c_sb

        def emit_loads_ot(b, eng):
            st = state[b]
            ot_sb = []
            for k in range(DT):
                otk = ot_pool.tile([128, Q], f32r, tag=f"ot{k}")
                eng.dma_start(otk[:], ot_in.ap()[b, 128 * k:128 * (k + 1), :])
                ot_sb.append(otk)
            st["ot"] = ot_sb

        def emit_ohT(b):
            # ohT [64, S] from oh on-device: 8 PE transposes of [128s, 64] + 2 copies
            st = state[b]
            oh = st["oh"]
            ohT = aux.tile([NSEG, S], f32r, tag="ohT")
            for g in range(2):
                po = ps_sm.tile([NSEG, 512], f32r, tag="pss")
                for h in range(4):
                    i = 4 * g + h
                    nc.tensor.transpose(po[0:NSEG, 128 * h:128 * (h + 1)],
                                        oh[:, NSEG * i:NSEG * (i + 1)], identr[:])
                nc.vector.tensor_copy(ohT[:, 512 * g:512 * (g + 1)], po[:])
            st["ohT"] = ohT

        def emit_front(b):
            st = state[b]
            oh, ohT, invc, cntb = st["oh"], st["ohT"], st["invc"], st["cntb"]
            ot_sb = st["ot"]

            # Csum[n, d] = sum_{s in seg n} C[s, d]  (2 psum halves)
            cs0 = ps_cs.tile([NSEG, 512], f32, tag="cs")
            cs1 = ps_cs.tile([NSEG, 512], f32, tag="cs")
            for i in range(ST):
                c_i = st["c"][i]
                nc.tensor.matmul(cs0[:], oh[:, NSEG * i:NSEG * (i + 1)],
                                 c_i[:, 0:512], start=(i == 0), stop=(i == ST - 1))
                nc.tensor.matmul(cs1[:], oh[:, NSEG * i:NSEG * (i + 1)],
                                 c_i[:, 512:1024], start=(i == 0), stop=(i == ST - 1))
            csum_sb = mid.tile([NSEG, D], f32r, tag="csum")
            nc.vector.tensor_copy(csum_sb[:, 0:512], cs0[:])
            nc.vector.tensor_copy(csum_sb[:, 512:1024], cs1[:])
            st["csum"] = csum_sb

            # CsumT: 8 transposes of [64,128] -> packed [128, 64*8]
            csumT_sb = mid1.tile([128, NSEG * DT], f32r, tag="csumT")
            for g in range(2):
                pss = ps_sm.tile([128, 256], f32r, tag="pss")
                for h in range(4):
                    d = 4 * g + h
                    nc.tensor.transpose(pss[:, 64 * h:64 * (h + 1)],
                                        csum_sb[0:NSEG, 128 * d:128 * (d + 1)],
                                        identr[0:NSEG, 0:NSEG])
                nc.vector.tensor_copy(csumT_sb[:, 256 * g:256 * (g + 1)], pss[:])

            # segavgT[n, q] = (Csum @ O^T)[n, q] * invc[n]
            sg = ps_cs.tile([NSEG, Q], f32, tag="cs")
            for d in range(DT):
                nc.tensor.matmul(sg[:], csumT_sb[:, NSEG * d:NSEG * (d + 1)],
                                 ot_sb[d][:], start=(d == 0), stop=(d == DT - 1))
            segavgT_sb = mid1.tile([NSEG, Q], f32, tag="segavgT")
            nc.vector.tensor_scalar_mul(segavgT_sb[:], sg[:], invc[:])

            # segavg [q, (j n)] via 4 transposes (exact fp32)
            pss2 = ps_sm.tile([128, QT * NSEG], f32, tag="pss")
            for j in range(QT):
                nc.tensor.transpose(pss2[:, NSEG * j:NSEG * (j + 1)],
                                    segavgT_sb[0:NSEG, 128 * j:128 * (j + 1)],
                                    ident[0:NSEG, 0:NSEG])
            segavg_sb = mid1.tile([128, QT * NSEG], f32, tag="segavg")
            nc.vector.tensor_copy(segavg_sb[:], pss2[:])

            # softmax over segments with exact counts
            mx = mid1.tile([128, QT], f32, tag="mx")
            nc.vector.reduce_max(mx[:], segavg_sb[:].rearrange("p (j n) -> p j n", n=NSEG),
                                 axis=mybir.AxisListType.X)
            neg_mx = mid1.tile([128, QT], f32, tag="neg_mx")
            nc.vector.tensor_scalar_mul(neg_mx[:], mx[:], -1.0)
            e_sb = mid1.tile([128, QT * NSEG], f32, tag="e")
            for j in range(QT):
                nc.scalar.activation(e_sb[:, NSEG * j:NSEG * (j + 1)],
                                     segavg_sb[:, NSEG * j:NSEG * (j + 1)],
                                     Exp, bias=neg_mx[:, j:j + 1])
            w_sb = mid1.tile([128, QT * NSEG], f32, tag="w")
            nc.vector.tensor_mul(w_sb[:], e_sb[:], cntb[:])
            dsum = mid1.tile([128, QT], f32, tag="dsum")
            nc.vector.reduce_sum(dsum[:], w_sb[:].rearrange("p (j n) -> p j n", n=NSEG),
                                 axis=mybir.AxisListType.X)
            rd = mid1.tile([128, QT], f32, tag="rd")
            nc.vector.reciprocal(rd[:], dsum[:])
            enorm_sb = mid1.tile([128, QT * NSEG], f32, tag="enorm")
            for j in range(QT):
                nc.vector.tensor_scalar_mul(enorm_sb[:, NSEG * j:NSEG * (j + 1)],
                                            e_sb[:, NSEG * j:NSEG * (j + 1)],
                                            rd[:, j:j + 1])

            # EnormT [n, q] via 4 transposes, converted to f32r
            pse = ps_sm.tile([NSEG, Q], f32, tag="pss")
            for j in range(QT):
                nc.tensor.transpose(pse[0:NSEG, 128 * j:128 * (j + 1)],
                                    enorm_sb[:, NSEG * j:NSEG * (j + 1)], ident[:])
            enT_sb = mid.tile([NSEG, Q], f32r, tag="enT")
            nc.vector.tensor_copy(enT_sb[:], pse[:])
            st["enT"] = enT_sb

        def emit_mid(b):
            st = state[b]
            ohT, csum_sb, enT_sb = st["ohT"], st["csum"], st["enT"]

            # align output: Enorm broadcast through one-hot^T
            for j in range(QT):
                for h in range(2):
                    pa = ps_mm.tile([128, 512], f32, tag="pmm")
                    nc.tensor.matmul(pa[:], enT_sb[0:NSEG, 128 * j:128 * (j + 1)],
                                     ohT[:, 512 * h:512 * (h + 1)],
                                     start=True, stop=True)
                    stg = stage.tile([128, 512], f32, tag="al_st")
                    nc.vector.tensor_copy(stg[:], pa[:])
                    nc.scalar.dma_start(
                        align_o.ap()[b, 128 * j:128 * (j + 1), 512 * h:512 * (h + 1)],
                        stg[:])

            # aligned-context^T tiles: ACT_d[dd, q] = sum_n Csum[n, dd]·EnormT[n, q]
            act_sb = []
            for d in range(DT):
                pm = ps_mm.tile([128, 512], f32, tag="pmm")
                nc.tensor.matmul(pm[:], csum_sb[0:NSEG, 128 * d:128 * (d + 1)],
                                 enT_sb[:], start=True, stop=True)
                actd = act_pool.tile([128, Q], f16, tag=f"act{d}")
                nc.vector.tensor_copy(actd[:], pm[:])
                act_sb.append(actd)
            st["act"] = act_sb

            # fp16 copy of O^T for the fp16 M3 pass
            ot16_sb = []
            for k in range(DT):
                o16 = act_pool.tile([128, Q], f16, tag=f"ot16_{k}")
                nc.vector.tensor_copy(o16[:], st["ot"][k][:])
                ot16_sb.append(o16)
            st["ot16"] = ot16_sb

        def emit_m3(b, eb, wt_map):
            st = state[b]
            act_sb, ot_sb = st["act"], st["ot16"]
            for j in range(QT):
                pm = ps_mm.tile([128, 512], f32, tag="pmm")
                nc.tensor.matmul(pm[:], ones_sb[:],
                                 biasr_sb[:, 512 * eb:512 * (eb + 1)],
                                 start=True, stop=False)
                for f in range(FT):
                    lhs = (act_sb[f] if f < DT else ot_sb[f - DT])
                    nc.tensor.matmul(pm[:], lhs[:, 128 * j:128 * (j + 1)],
                                     wt_map[(eb, f)][:],
                                     start=False, stop=(f == FT - 1))
                ost = stage.tile([128, 512], f32, tag="out_st")
                nc.scalar.activation(ost[:], pm[:], Tanh)
                nc.scalar.dma_start(
                    out_o.ap()[b, 128 * j:128 * (j + 1), 512 * eb:512 * (eb + 1)],
                    ost[:])

        def emit_wt(eb):
            for f in range(FT):
                w = wt_pool.tile([128, D // 2], f16, tag=f"wt{eb}_{f}")
                nc.sync.dma_start(w[:], wt_in.ap()[eb, 128 * f:128 * (f + 1), :])
                wt_map[(eb, f)] = w

        wt_map = {}
        # Ring plan: Sync = aux0, C0, WTe0, OT1, WTe1; Scalar = OT0 + outputs;
        # GpSimd = aux1 + slot-gated C1.
        emit_loads_aux(0, nc.sync)
        emit_loads_ot(0, nc.scalar)
        emit_loads_c(0, nc.sync)
        emit_loads_aux(1, nc.gpsimd)
        emit_loads_c(1, nc.gpsimd)
        emit_loads_ot(1, nc.sync)
        emit_wt(0)
        emit_wt(1)
        nc.sync.dma_start(biasr_sb[:], biasr_in.ap())
        nc.sync.dma_start(ones_sb[:], ones_in.ap())

        emit_ohT(0)
        emit_front(0)
        emit_mid(0)
        emit_m3(0, 0, wt_map)
        emit_ohT(1)
        emit_front(1)
        emit_mid(1)
        emit_m3(1, 0, wt_map)
        emit_m3(0, 1, wt_map)
        emit_m3(1, 1, wt_map)

    nc.compile()
    return nc


def _host_prep(output, context, W_weight, W_bias, segment_ids):
    """Shard over batch + build per-core input maps (host-side index prep)."""
    wt_full = W_weight.T.astype(np.float16)                            # [2D, D] fp16
    wt = np.ascontiguousarray(
        np.stack([wt_full[:, :D // 2], wt_full[:, D // 2:]]))          # [2, 2D, D/2]
    biasr = np.ascontiguousarray(W_bias.astype(np.float16)[None, :])
    ones = np.ones((1, 128), dtype=np.float16)
    ident = np.eye(128, dtype=np.float32)

    in_maps = []
    for c in range(NCORES):
        lo = c * BPC
        ohs, invcs, cntbs = [], [], []
        for b in range(BPC):
            ids = segment_ids[lo + b].astype(np.int32)                # [S]
            oh = (ids[:, None] == np.arange(NSEG, dtype=np.int32)[None, :]
                  ).astype(np.float32)                                # [S, NSEG]
            cnt = oh.sum(axis=0)                                      # [NSEG]
            inv = (1.0 / np.maximum(cnt, 1.0)).astype(np.float32)
            oh_packed = np.ascontiguousarray(
                oh.reshape(ST, 128, NSEG).transpose(1, 0, 2).reshape(128, ST * NSEG))
            ohs.append(oh_packed)
            invcs.append(inv[:, None])
            cntbs.append(np.ascontiguousarray(
                np.broadcast_to(np.tile(cnt.astype(np.float32), QT)[None, :],
                                (128, QT * NSEG))))
        in_maps.append({
            "ot_in": np.ascontiguousarray(
                output[lo:lo + BPC].astype(np.float32).transpose(0, 2, 1)),
            "context_in": np.ascontiguousarray(context[lo:lo + BPC].astype(np.float32)),
            "wt_in": wt,
            "biasr_in": biasr,
            "ones_in": ones,
            "ident_in": ident,
            "identr_in": ident,
            "oh_in": np.stack(ohs),
            "invc_in": np.stack(invcs),
            "cntb_in": np.stack(cntbs),
        })
    return in_maps


def _run(inputs, trace=False, tmpdir=None):
    from concourse.bass_utils import run_bass_kernel_spmd
    if "nc" not in _CACHE:
        _CACHE["nc"] = _build_nc()
    nc = _CACHE["nc"]
    in_maps = _host_prep(**inputs)
    kw = {}
    if trace:
        kw = {"trace": True, "tmpdir": tmpdir}
    res = run_bass_kernel_spmd(nc, in_maps, core_ids=list(range(NCORES)), **kw)
    out = np.concatenate([res.results[c]["out_o"] for c in range(NCORES)], axis=0)
    align = np.concatenate([res.results[c]["align_o"] for c in range(NCORES)], axis=0)
    return (out, align), res


def kernel(output, context, W_weight, W_bias, segment_ids):
    # Force host numpy up front: if the caller hands us jax arrays, numpy
    # ops would otherwise dispatch to the accelerator backend.
    (out, align), _ = _run(dict(
        output=np.asarray(output, dtype=np.float32),
        context=np.asarray(context, dtype=np.float32),
        W_weight=np.asarray(W_weight, dtype=np.float32),
        W_bias=np.asarray(W_bias, dtype=np.float32),
        segment_ids=np.asarray(segment_ids, dtype=np.int32)))
    return out, align



# revision 2
# speedup vs baseline: 1.4327x; 1.4327x over previous
"""Trainium2 Bass kernel for nn_Attn_30820685316537 (segment_reduce attention).

Reference computation (per batch b):
    score = output @ context^T                     [Q, S]
    avg   = per-segment mean of score over S, broadcast back
    align = softmax(avg, axis=S)                   [Q, S]
    ac    = align @ context                        [Q, D]
    out   = tanh(concat(ac, output) @ W^T + bias)  [Q, D]
    returns (out, align)

Algebraic structure exploited on device: `avg` is constant within each of the
64 contiguous segments, so the whole pipeline factors through rank-64 segment
space.  With Csum[n, d] = sum_{s in segment n} context[s, d]:
    segavg[q, n]  = (output[q, :] . Csum[n, :]) / max(cnt[n], 1)
    Enorm[q, n]   = softmax weights per segment (exp/sum with exact counts)
    align[q, s]   = Enorm[q, seg[s]]        (broadcast via 0/1 one-hot matmul)
and additionally the final projection's aligned-context half also factors:
    ac @ W1^T     = Enorm @ (Csum @ W1^T)   (CW := Csum @ W1^T is [64, D])
so aligned_context is never materialised; the M3 GEMM shrinks from
[Q,2D]@[2D,D] to [Q,D]@[D,D] (the output half) plus two rank-64 products.

Sharding: data-parallel over batch B=16 across 8 NeuronCores (2 batches per
core); W replicated.  All heavy streams are fp16 on the DMA path (PE rate is
dtype-independent; DMA bytes halve); the softmax middle section stays fp32.
A burst of warm-up matmuls at kernel start flips the PE HAM clock-gate to
full rate while the first context DMA is still in flight, and the emission
order keeps the PE dense (2 batches software-pipelined) so it stays warm.
"""
import numpy as np
from contextlib import ExitStack

B, Q, S, D = 16, 512, 1024, 1024
NSEG = 64
NCORES = 8
BPC = B // NCORES          # batches per core
QT = Q // 128              # 4 q-tiles
ST = S // 128              # 8 s-tiles
DT = D // 128              # 8 d-tiles

_CACHE = {}


def _wt_col(eb, f):
    """Column offset of W^T tile (eb, f) in the packed wt_sb layout.

    Tiles with f<8 (the CW / aligned-context half) are packed first so their
    DMA can land early; f>=8 (the output half) follows."""
    if f < DT:
        return (eb * DT + f) * 512
    return (2 * DT + eb * DT + (f - DT)) * 512


def _build_nc():
    import concourse.bacc as bacc
    import concourse.tile as tile
    import concourse.mybir as mybir

    f32 = mybir.dt.float32
    f32r = mybir.dt.float32r
    f16 = mybir.dt.float16

    nc = bacc.Bacc("TRN2", target_bir_lowering=False, debug=False,
                   enable_asserts=False, num_devices=NCORES)

    ident_in = nc.dram_tensor("ident_in", [128, 128], f32, kind="ExternalInput")
    identr_in = nc.dram_tensor("identr_in", [128, 128], f32r, kind="ExternalInput")
    c_in = nc.dram_tensor("c_in", [BPC, 128, ST * D], f16, kind="ExternalInput")
    ot_in = nc.dram_tensor("ot_in", [BPC, 128, DT * Q], f16, kind="ExternalInput")
    wt_in = nc.dram_tensor("wt_in", [128, 32 * 512], f16, kind="ExternalInput")
    ohp_in = nc.dram_tensor("ohp_in", [BPC, 128, ST * NSEG], f16, kind="ExternalInput")
    ohT_in = nc.dram_tensor("ohT_in", [BPC, NSEG, S], f16, kind="ExternalInput")
    cntb_in = nc.dram_tensor("cntb_in", [BPC, 128, QT * NSEG], f32, kind="ExternalInput")
    invc_in = nc.dram_tensor("invc_in", [BPC, NSEG, 1], f32, kind="ExternalInput")
    biasr_in = nc.dram_tensor("biasr_in", [1, D], f16, kind="ExternalInput")

    out_o = nc.dram_tensor("out_o", [BPC, Q, D], f16, kind="ExternalOutput")
    align_o = nc.dram_tensor("align_o", [BPC, Q, S], f16, kind="ExternalOutput")

    Exp = mybir.ActivationFunctionType.Exp
    Tanh = mybir.ActivationFunctionType.Tanh

    with tile.TileContext(nc) as tc, ExitStack() as ctx:
        consts = ctx.enter_context(tc.tile_pool(name="consts", bufs=1))
        wt_pool = ctx.enter_context(tc.tile_pool(name="wt", bufs=1))
        c_pool = ctx.enter_context(tc.tile_pool(name="cp", bufs=2))
        ot_pool = ctx.enter_context(tc.tile_pool(name="otp", bufs=2))
        aux = ctx.enter_context(tc.tile_pool(name="aux", bufs=2))
        mid = ctx.enter_context(tc.tile_pool(name="mid", bufs=2))
        fr = ctx.enter_context(tc.tile_pool(name="fr", bufs=2))
        stage = ctx.enter_context(tc.tile_pool(name="stage", bufs=3))

        ps_a = ctx.enter_context(tc.tile_pool(name="ps_a", bufs=3, space="PSUM"))
        ps_al = ctx.enter_context(tc.tile_pool(name="ps_al", bufs=2, space="PSUM"))
        ps_o = ctx.enter_context(tc.tile_pool(name="ps_o", bufs=3, space="PSUM"))

        # ---- constants / weights ----
        ident = consts.tile([128, 128], f32, tag="ident")
        nc.sync.dma_start(ident[:], ident_in.ap())
        identr = consts.tile([128, 128], f32r, tag="identr")
        nc.sync.dma_start(identr[:], identr_in.ap())
        biasr_sb = consts.tile([1, D], f16, tag="biasr")
        nc.gpsimd.dma_start(biasr_sb[:], biasr_in.ap())
        wt_sb = wt_pool.tile([128, 32 * 512], f16, tag="wt")
        # low half (f<8, both eb) needed first for CW; high half for M3
        nc.scalar.dma_start(wt_sb[:, 0:16 * 512], wt_in.ap()[:, 0:16 * 512])

        state = [dict() for _ in range(BPC)]

        def emit_loads_aux(b, eng):
            st = state[b]
            ohp = aux.tile([128, ST * NSEG], f16, tag="ohp")
            eng.dma_start(ohp[:], ohp_in.ap()[b])
            ohT = aux.tile([NSEG, S], f16, tag="ohT")
            eng.dma_start(ohT[:], ohT_in.ap()[b])
            cntb = aux.tile([128, QT * NSEG], f32, tag="cntb")
            eng.dma_start(cntb[:], cntb_in.ap()[b])
            invc = aux.tile([NSEG, 1], f32, tag="invc")
            eng.dma_start(invc[:], invc_in.ap()[b])
            st["ohp"], st["ohT"], st["cntb"], st["invc"] = ohp, ohT, cntb, invc

        def emit_loads_c(b, eng):
            c_sb = c_pool.tile([128, ST * D], f16, tag="c")
            eng.dma_start(c_sb[:], c_in.ap()[b])
            state[b]["c"] = c_sb

        def emit_loads_ot(b, eng):
            ot_sb = ot_pool.tile([128, DT * Q], f16, tag="ot")
            eng.dma_start(ot_sb[:], ot_in.ap()[b])
            state[b]["ot"] = ot_sb

        def emit_warmup(n_mm):
            # Real matmuls (transposes don't count as PE-busy for the HAM
            # clock gate): flip the PE to 2.4 GHz while input DMAs stream.
            wps = ps_o.tile([128, 512], f32, tag="po")
            for _ in range(n_mm):
                nc.tensor.matmul(wps[:, 0:128], identr[:], identr[:],
                                 start=True, stop=True)

        def emit_csum(b):
            # Csum[n, d] = sum_{s in seg n} C[s, d]
            st = state[b]
            ohp, c_sb = st["ohp"], st["c"]
            cs0 = ps_a.tile([NSEG, 512], f32, tag="a")
            cs1 = ps_a.tile([NSEG, 512], f32, tag="a")
            for i in range(ST):
                nc.tensor.matmul(cs0[:], ohp[:, NSEG * i:NSEG * (i + 1)],
                                 c_sb[:, i * D:i * D + 512],
                                 start=(i == 0), stop=(i == ST - 1))
                nc.tensor.matmul(cs1[:], ohp[:, NSEG * i:NSEG * (i + 1)],
                                 c_sb[:, i * D + 512:i * D + 1024],
                                 start=(i == 0), stop=(i == ST - 1))
            csum_sb = fr.tile([NSEG, D], f32r, tag="csum")
            nc.vector.tensor_copy(csum_sb[:, 0:512], cs0[:])
            nc.vector.tensor_copy(csum_sb[:, 512:1024], cs1[:])
            st["csum"] = csum_sb

        def emit_csumT(b):
            # csumT [128, (d n)] fp16 via 8 PE transposes of [64, 128] blocks
            st = state[b]
            csum_sb = st["csum"]
            pt = ps_a.tile([128, NSEG * DT], f32r, tag="a")
            for d in range(DT):
                nc.tensor.transpose(pt[:, NSEG * d:NSEG * (d + 1)],
                                    csum_sb[0:NSEG, 128 * d:128 * (d + 1)],
                                    identr[0:NSEG, 0:NSEG])
            csumT_sb = mid.tile([128, NSEG * DT], f16, tag="csumT")
            nc.vector.tensor_copy(csumT_sb[:], pt[:])
            st["csumT"] = csumT_sb

        def emit_segavgT(b):
            # segavgT[n, q] = (Csum @ O^T)[n, q] * invc[n]
            st = state[b]
            csumT_sb, ot_sb, invc = st["csumT"], st["ot"], st["invc"]
            sg = ps_a.tile([NSEG, Q], f32, tag="a")
            for d in range(DT):
                nc.tensor.matmul(sg[:], csumT_sb[:, NSEG * d:NSEG * (d + 1)],
                                 ot_sb[:, d * Q:(d + 1) * Q],
                                 start=(d == 0), stop=(d == DT - 1))
            segavgT_sb = fr.tile([NSEG, Q], f32, tag="segavgT")
            nc.vector.tensor_scalar_mul(segavgT_sb[:], sg[:], invc[:])
            st["segavgT"] = segavgT_sb

        def emit_segavg(b):
            # segavg [q, (j n)] via 4 exact fp32 transposes
            st = state[b]
            pt = ps_a.tile([128, QT * NSEG], f32, tag="a")
            for j in range(QT):
                nc.tensor.transpose(pt[:, NSEG * j:NSEG * (j + 1)],
                                    st["segavgT"][0:NSEG, 128 * j:128 * (j + 1)],
                                    ident[0:NSEG, 0:NSEG])
            segavg_sb = fr.tile([128, QT * NSEG], f32, tag="segavg")
            nc.vector.tensor_copy(segavg_sb[:], pt[:])
            st["segavg"] = segavg_sb

        def emit_softmax(b):
            # softmax over segments with exact counts (all fp32)
            st = state[b]
            segavg_sb, cntb = st["segavg"], st["cntb"]
            mx = fr.tile([128, QT], f32, tag="mx")
            nc.vector.reduce_max(mx[:], segavg_sb[:].rearrange("p (j n) -> p j n", n=NSEG),
                                 axis=mybir.AxisListType.X)
            neg_mx = fr.tile([128, QT], f32, tag="neg_mx")
            nc.vector.tensor_scalar_mul(neg_mx[:], mx[:], -1.0)
            e_sb = fr.tile([128, QT * NSEG], f32, tag="e")
            for j in range(QT):
                nc.scalar.activation(e_sb[:, NSEG * j:NSEG * (j + 1)],
                                     segavg_sb[:, NSEG * j:NSEG * (j + 1)],
                                     Exp, bias=neg_mx[:, j:j + 1])
            w_sb = fr.tile([128, QT * NSEG], f32, tag="w")
            nc.vector.tensor_mul(w_sb[:], e_sb[:], cntb[:])
            dsum = fr.tile([128, QT], f32, tag="dsum")
            nc.vector.reduce_sum(dsum[:], w_sb[:].rearrange("p (j n) -> p j n", n=NSEG),
                                 axis=mybir.AxisListType.X)
            rd = fr.tile([128, QT], f32, tag="rd")
            nc.vector.reciprocal(rd[:], dsum[:])
            enorm_sb = fr.tile([128, QT * NSEG], f32, tag="enorm")
            for j in range(QT):
                nc.vector.tensor_scalar_mul(enorm_sb[:, NSEG * j:NSEG * (j + 1)],
                                            e_sb[:, NSEG * j:NSEG * (j + 1)],
                                            rd[:, j:j + 1])
            st["enorm"] = enorm_sb

        def emit_enT(b):
            # EnormT [n, q] fp16 + a row of ones (row 64) for the fused bias
            st = state[b]
            pe = ps_a.tile([NSEG, Q], f32, tag="a")
            for j in range(QT):
                nc.tensor.transpose(pe[0:NSEG, 128 * j:128 * (j + 1)],
                                    st["enorm"][:, NSEG * j:NSEG * (j + 1)], ident[:])
            enT_sb = mid.tile([NSEG + 1, Q], f16, tag="enT")
            nc.vector.tensor_copy(enT_sb[0:NSEG, :], pe[:])
            nc.vector.memset(enT_sb[NSEG:NSEG + 1, :], 1.0)
            st["enT"] = enT_sb

        def emit_cw(b):
            # CW[n, c] = (Csum @ W1^T)[n, c]; row 64 = bias (for the K=65 MM)
            st = state[b]
            csumT_sb = st["csumT"]
            cw = []
            for eb in range(2):
                pcw = ps_a.tile([NSEG, 512], f32, tag="a")
                for d in range(DT):
                    nc.tensor.matmul(pcw[:], csumT_sb[:, NSEG * d:NSEG * (d + 1)],
                                     wt_sb[:, _wt_col(eb, d):_wt_col(eb, d) + 512],
                                     start=(d == 0), stop=(d == DT - 1))
                cw_sb = mid.tile([NSEG + 1, 512], f16, tag=f"cw{eb}")
                nc.vector.tensor_copy(cw_sb[0:NSEG, :], pcw[:])
                nc.vector.tensor_copy(cw_sb[NSEG:NSEG + 1, :],
                                      biasr_sb[0:1, 512 * eb:512 * (eb + 1)])
                cw.append(cw_sb)
            st["cw"] = cw

        def emit_align(b, out_eng):
            # align[q, s] = Enorm[q, seg[s]] via one-hot^T broadcast matmul
            st = state[b]
            enT_sb, ohT = st["enT"], st["ohT"]
            for j in range(QT):
                stg = stage.tile([128, S], f16, tag="al_st")
                for h in range(2):
                    pa = ps_al.tile([128, 512], f32, tag="al")
                    nc.tensor.matmul(pa[:], enT_sb[0:NSEG, 128 * j:128 * (j + 1)],
                                     ohT[:, 512 * h:512 * (h + 1)],
                                     start=True, stop=True)
                    nc.vector.tensor_copy(stg[:, 512 * h:512 * (h + 1)], pa[:])
                out_eng.dma_start(align_o.ap()[b, 128 * j:128 * (j + 1), :], stg[:])

        def emit_m3(b, out_eng):
            # out = tanh(O @ W2^T + Enorm @ CW)   (CW already carries the bias)
            st = state[b]
            ot_sb, enT_sb, cw = st["ot"], st["enT"], st["cw"]
            for j in range(QT):
                ost = stage.tile([128, D], f16, tag="out_st")
                for eb in range(2):
                    po = ps_o.tile([128, 512], f32, tag="po")
                    for f in range(DT):
                        nc.tensor.matmul(po[:],
                                         ot_sb[:, f * Q + 128 * j:f * Q + 128 * (j + 1)],
                                         wt_sb[:, _wt_col(eb, DT + f):_wt_col(eb, DT + f) + 512],
                                         start=(f == 0), stop=False)
                    nc.tensor.matmul(po[:], enT_sb[:, 128 * j:128 * (j + 1)],
                                     cw[eb][:], start=False, stop=True)
                    nc.scalar.activation(ost[:, 512 * eb:512 * (eb + 1)], po[:], Tanh)
                out_eng.dma_start(out_o.ap()[b, 128 * j:128 * (j + 1), :], ost[:])

        # ---- DMA issue plan ----
        # sync:   ident/identr, c0, c1      scalar: ot0, wt_lo, ot1, wt_hi, out
        # gpsimd: aux + align out
        emit_loads_aux(0, nc.gpsimd)
        emit_loads_ot(0, nc.scalar)
        emit_loads_c(0, nc.sync)
        emit_loads_c(1, nc.sync)
        emit_loads_ot(1, nc.scalar)
        emit_loads_aux(1, nc.gpsimd)
        nc.scalar.dma_start(wt_sb[:, 16 * 512:32 * 512], wt_in.ap()[:, 16 * 512:32 * 512])

        # ---- compute schedule (PE kept dense; 2 batches software-pipelined) ----
        emit_warmup(40)
        emit_csum(0)
        emit_csum(1)
        emit_csumT(0)
        emit_csumT(1)
        emit_segavgT(0)
        emit_segavg(0)
        emit_softmax(0)
        emit_segavgT(1)
        emit_segavg(1)
        emit_softmax(1)
        emit_cw(0)
        emit_enT(0)
        emit_cw(1)
        emit_align(0, nc.gpsimd)
        emit_m3(0, nc.scalar)
        emit_enT(1)
        emit_align(1, nc.gpsimd)
        emit_m3(1, nc.scalar)

    nc.compile()
    return nc


def _host_prep(output, context, W_weight, W_bias, segment_ids):
    """Shard over batch + build per-core input maps (host-side packing)."""
    wt_full = W_weight.T.astype(np.float16)                            # [2D, D]
    wtp = np.empty((128, 32 * 512), dtype=np.float16)
    for eb in range(2):
        for f in range(16):
            col = _wt_col(eb, f)
            wtp[:, col:col + 512] = wt_full[128 * f:128 * (f + 1),
                                            512 * eb:512 * (eb + 1)]
    biasr = np.ascontiguousarray(W_bias.astype(np.float16)[None, :])
    ident = np.eye(128, dtype=np.float32)

    in_maps = []
    for c in range(NCORES):
        lo = c * BPC
        cs, ots, ohps, ohTs, cntbs, invcs = [], [], [], [], [], []
        for b in range(BPC):
            ctx = context[lo + b].astype(np.float16)                  # [S, D]
            cs.append(ctx.reshape(ST, 128, D).transpose(1, 0, 2).reshape(128, ST * D))
            otb = output[lo + b].T.astype(np.float16)                 # [D, Q]
            ots.append(otb.reshape(DT, 128, Q).transpose(1, 0, 2).reshape(128, DT * Q))
            ids = segment_ids[lo + b].astype(np.int32)                # [S]
            oh = (ids[:, None] == np.arange(NSEG, dtype=np.int32)[None, :])
            ohf = oh.astype(np.float16)                               # [S, NSEG]
            cnt = oh.sum(axis=0).astype(np.float32)                   # [NSEG]
            ohps.append(np.ascontiguousarray(
                ohf.reshape(ST, 128, NSEG).transpose(1, 0, 2).reshape(128, ST * NSEG)))
            ohTs.append(np.ascontiguousarray(ohf.T))                  # [NSEG, S]
            cntbs.append(np.ascontiguousarray(
                np.broadcast_to(np.tile(cnt, QT)[None, :], (128, QT * NSEG))))
            invcs.append((1.0 / np.maximum(cnt, 1.0)).astype(np.float32)[:, None])
        in_maps.append({
            "ident_in": ident,
            "identr_in": ident,
            "c_in": np.ascontiguousarray(np.stack(cs)),
            "ot_in": np.ascontiguousarray(np.stack(ots)),
            "wt_in": wtp,
            "ohp_in": np.stack(ohps),
            "ohT_in": np.stack(ohTs),
            "cntb_in": np.stack(cntbs),
            "invc_in": np.stack(invcs),
            "biasr_in": biasr,
        })
    return in_maps


def _run(inputs, trace=False, tmpdir=None):
    from concourse.bass_utils import run_bass_kernel_spmd
    if "nc" not in _CACHE:
        _CACHE["nc"] = _build_nc()
    nc = _CACHE["nc"]
    in_maps = _host_prep(**inputs)
    kw = {}
    if trace:
        kw = {"trace": True, "tmpdir": tmpdir}
    res = run_bass_kernel_spmd(nc, in_maps, core_ids=list(range(NCORES)), **kw)
    out = np.concatenate([res.results[c]["out_o"] for c in range(NCORES)],
                         axis=0).astype(np.float32)
    align = np.concatenate([res.results[c]["align_o"] for c in range(NCORES)],
                           axis=0).astype(np.float32)
    return (out, align), res


def kernel(output, context, W_weight, W_bias, segment_ids):
    # Force host numpy up front: if the caller hands us jax arrays, numpy
    # ops would otherwise dispatch to the accelerator backend.
    (out, align), _ = _run(dict(
        output=np.asarray(output, dtype=np.float32),
        context=np.asarray(context, dtype=np.float32),
        W_weight=np.asarray(W_weight, dtype=np.float32),
        W_bias=np.asarray(W_bias, dtype=np.float32),
        segment_ids=np.asarray(segment_ids, dtype=np.int32)))
    return out, align


# revision 12
# speedup vs baseline: 1.7153x; 1.1973x over previous
"""Trainium2 Bass kernel for nn_Attn_30820685316537 (segment_reduce attention).

Reference computation (per batch b):
    score = output @ context^T                     [Q, S]
    avg   = per-segment mean of score over S, broadcast back
    align = softmax(avg, axis=S)                   [Q, S]
    ac    = align @ context                        [Q, D]
    out   = tanh(concat(ac, output) @ W^T + bias)  [Q, D]
    returns (out, align)

Algebraic structure exploited on device: `avg` is constant within each of the
64 contiguous segments, so the whole pipeline factors through rank-64 segment
space.  With Csum[n, d] = sum_{s in segment n} context[s, d]:
    segavg[q, n]  = (output[q, :] . Csum[n, :]) / max(cnt[n], 1)
    Enorm[q, n]   = softmax weights per segment (exp/sum with exact counts)
    align[q, s]   = Enorm[q, seg[s]]        (broadcast via 0/1 one-hot matmul)
and additionally the final projection's aligned-context half also factors:
    ac @ W1^T     = Enorm @ (Csum @ W1^T)   (CW := Csum @ W1^T is [64, D])
so aligned_context is never materialised; the M3 GEMM shrinks from
[Q,2D]@[2D,D] to [Q,D]@[D,D] (the output half) plus two rank-64 products.

Sharding: data-parallel over batch B=16 across 8 NeuronCores (2 batches per
core); W replicated.  All heavy streams are fp16 on the DMA path (PE rate is
dtype-independent; DMA bytes halve); the softmax middle section stays fp32.
A burst of warm-up matmuls at kernel start flips the PE HAM clock-gate to
full rate while the first context DMA is still in flight, and the emission
order keeps the PE dense (2 batches software-pipelined) so it stays warm.
"""
import numpy as np
from contextlib import ExitStack

B, Q, S, D = 16, 512, 1024, 1024
NSEG = 64
NCORES = 8
BPC = B // NCORES          # batches per core
QT = Q // 128              # 4 q-tiles
ST = S // 128              # 8 s-tiles
DT = D // 128              # 8 d-tiles

_CACHE = {}


def _wt_col(eb, f):
    """Column offset of W^T tile (eb, f) inside its packed half (lo: f<8 —
    the CW half; hi: f>=8 — the output half)."""
    return (eb * DT + f % DT) * 512


def _build_nc():
    import concourse.bacc as bacc
    import concourse.tile as tile
    import concourse.mybir as mybir

    f32 = mybir.dt.float32
    f32r = mybir.dt.float32r
    f16 = mybir.dt.float16

    nc = bacc.Bacc("TRN2", target_bir_lowering=False, debug=False,
                   enable_asserts=False, num_devices=NCORES)

    ident_in = nc.dram_tensor("ident_in", [128, 128], f32, kind="ExternalInput")
    identr_in = nc.dram_tensor("identr_in", [128, 128], f32r, kind="ExternalInput")
    c_in = nc.dram_tensor("c_in", [BPC, 128, ST * D], f16, kind="ExternalInput")
    ot_in = nc.dram_tensor("ot_in", [BPC, 128, DT * Q], f16, kind="ExternalInput")
    wtlo_in = nc.dram_tensor("wtlo_in", [128, 16 * 512], f16, kind="ExternalInput")
    wthi_in = nc.dram_tensor("wthi_in", [128, 16 * 512], f16, kind="ExternalInput")
    ohp_in = nc.dram_tensor("ohp_in", [BPC, 128, ST * NSEG], f16, kind="ExternalInput")
    ohT_in = nc.dram_tensor("ohT_in", [BPC, NSEG, S], f16, kind="ExternalInput")
    cntb_in = nc.dram_tensor("cntb_in", [BPC, 128, QT * NSEG], f32, kind="ExternalInput")
    invc_in = nc.dram_tensor("invc_in", [BPC, NSEG, 1], f32, kind="ExternalInput")
    biasr_in = nc.dram_tensor("biasr_in", [1, D], f16, kind="ExternalInput")

    out_o = nc.dram_tensor("out_o", [BPC, Q, D], f16, kind="ExternalOutput")
    align_o = nc.dram_tensor("align_o", [BPC, Q, S], f16, kind="ExternalOutput")

    Exp = mybir.ActivationFunctionType.Exp
    Tanh = mybir.ActivationFunctionType.Tanh

    with tile.TileContext(nc) as tc, ExitStack() as ctx:
        consts = ctx.enter_context(tc.tile_pool(name="consts", bufs=1))
        wt_pool = ctx.enter_context(tc.tile_pool(name="wt", bufs=1))
        c_pool = ctx.enter_context(tc.tile_pool(name="cp", bufs=2))
        ot_pool = ctx.enter_context(tc.tile_pool(name="otp", bufs=2))
        aux = ctx.enter_context(tc.tile_pool(name="aux", bufs=2))
        mid = ctx.enter_context(tc.tile_pool(name="mid", bufs=2))
        fr = ctx.enter_context(tc.tile_pool(name="fr", bufs=2))
        stage = ctx.enter_context(tc.tile_pool(name="stage", bufs=3))

        ps_a = ctx.enter_context(tc.tile_pool(name="ps_a", bufs=3, space="PSUM"))
        ps_al = ctx.enter_context(tc.tile_pool(name="ps_al", bufs=2, space="PSUM"))
        ps_o = ctx.enter_context(tc.tile_pool(name="ps_o", bufs=3, space="PSUM"))

        # ---- constants / weights ----
        ident = consts.tile([128, 128], f32, tag="ident")
        identr = consts.tile([128, 128], f32r, tag="identr")
        biasr_sb = consts.tile([1, D], f16, tag="biasr")
        nc.gpsimd.dma_start(biasr_sb[:], biasr_in.ap())
        wtlo_sb = wt_pool.tile([128, 16 * 512], f16, tag="wtlo")
        wthi_sb = wt_pool.tile([128, 16 * 512], f16, tag="wthi")
        junk = consts.tile([128, 128], f16, tag="junk")

        state = [dict() for _ in range(BPC)]

        def emit_loads_aux(b, eng):
            st = state[b]
            ohp = aux.tile([128, ST * NSEG], f16, tag="ohp")
            eng.dma_start(ohp[:], ohp_in.ap()[b])
            ohT = aux.tile([NSEG, S], f16, tag="ohT")
            eng.dma_start(ohT[:], ohT_in.ap()[b])
            cntb = aux.tile([128, QT * NSEG], f32, tag="cntb")
            eng.dma_start(cntb[:], cntb_in.ap()[b])
            invc = aux.tile([NSEG, 1], f32, tag="invc")
            eng.dma_start(invc[:], invc_in.ap()[b])
            st["ohp"], st["ohT"], st["cntb"], st["invc"] = ohp, ohT, cntb, invc

        def emit_loads_c(b, eng):
            c_sb = c_pool.tile([128, ST * D], f16, tag="c")
            eng.dma_start(c_sb[:], c_in.ap()[b])
            state[b]["c"] = c_sb

        def emit_loads_ot(b, eng):
            ot_sb = ot_pool.tile([128, DT * Q], f16, tag="ot")
            eng.dma_start(ot_sb[:], ot_in.ap()[b])
            state[b]["ot"] = ot_sb

        def emit_warmup(n_small, n_big):
            # Real matmuls on a memset tile (no DMA dependency; transposes
            # don't count as PE-busy for the HAM clock gate): flip the PE to
            # 2.4 GHz and keep it busy while the first input DMAs stream.
            nc.vector.memset(junk[:], 1.0)
            wps = ps_o.tile([128, 512], f32, tag="po")
            for _ in range(n_small):
                nc.tensor.matmul(wps[:, 0:128], junk[:], junk[:],
                                 start=True, stop=True)
            for _ in range(n_big):
                for h in range(4):
                    nc.tensor.matmul(wps[:, 128 * h:128 * (h + 1)],
                                     junk[:], junk[:], start=True, stop=True)

        def emit_csum(b):
            # Csum[n, d] = sum_{s in seg n} C[s, d]
            st = state[b]
            ohp, c_sb = st["ohp"], st["c"]
            cs0 = ps_a.tile([NSEG, 512], f32, tag="a")
            cs1 = ps_a.tile([NSEG, 512], f32, tag="a")
            for i in range(ST):
                nc.tensor.matmul(cs0[:], ohp[:, NSEG * i:NSEG * (i + 1)],
                                 c_sb[:, i * D:i * D + 512],
                                 start=(i == 0), stop=(i == ST - 1))
                nc.tensor.matmul(cs1[:], ohp[:, NSEG * i:NSEG * (i + 1)],
                                 c_sb[:, i * D + 512:i * D + 1024],
                                 start=(i == 0), stop=(i == ST - 1))
            csum_sb = fr.tile([NSEG, D], f32r, tag="csum")
            nc.vector.tensor_copy(csum_sb[:, 0:512], cs0[:])
            nc.vector.tensor_copy(csum_sb[:, 512:1024], cs1[:])
            st["csum"] = csum_sb

        def emit_csumT(b):
            # csumT [128, (d n)] fp16 via 8 PE transposes of [64, 128] blocks
            st = state[b]
            csum_sb = st["csum"]
            pt = ps_a.tile([128, NSEG * DT], f32r, tag="a")
            for d in range(DT):
                nc.tensor.transpose(pt[:, NSEG * d:NSEG * (d + 1)],
                                    csum_sb[0:NSEG, 128 * d:128 * (d + 1)],
                                    identr[0:NSEG, 0:NSEG])
            csumT_sb = mid.tile([128, NSEG * DT], f16, tag="csumT")
            nc.vector.tensor_copy(csumT_sb[:], pt[:])
            st["csumT"] = csumT_sb

        def emit_segavgT(b):
            # segavgT[n, q] = (Csum @ O^T)[n, q] * invc[n]
            st = state[b]
            csumT_sb, ot_sb, invc = st["csumT"], st["ot"], st["invc"]
            sg = ps_a.tile([NSEG, Q], f32, tag="a")
            for d in range(DT):
                nc.tensor.matmul(sg[:], csumT_sb[:, NSEG * d:NSEG * (d + 1)],
                                 ot_sb[:, d * Q:(d + 1) * Q],
                                 start=(d == 0), stop=(d == DT - 1))
            segavgT_sb = fr.tile([NSEG, Q], f32, tag="segavgT")
            nc.vector.tensor_scalar_mul(segavgT_sb[:], sg[:], invc[:])
            st["segavgT"] = segavgT_sb

        def emit_segavg(b):
            # segavg [q, (j n)] via 4 exact fp32 transposes
            st = state[b]
            pt = ps_a.tile([128, QT * NSEG], f32, tag="a")
            for j in range(QT):
                nc.tensor.transpose(pt[:, NSEG * j:NSEG * (j + 1)],
                                    st["segavgT"][0:NSEG, 128 * j:128 * (j + 1)],
                                    ident[0:NSEG, 0:NSEG])
            segavg_sb = fr.tile([128, QT * NSEG], f32, tag="segavg")
            nc.vector.tensor_copy(segavg_sb[:], pt[:])
            st["segavg"] = segavg_sb

        def emit_softmax(b):
            # softmax over segments with exact counts (all fp32)
            st = state[b]
            segavg_sb, cntb = st["segavg"], st["cntb"]
            mx = fr.tile([128, QT], f32, tag="mx")
            nc.vector.reduce_max(mx[:], segavg_sb[:].rearrange("p (j n) -> p j n", n=NSEG),
                                 axis=mybir.AxisListType.X)
            neg_mx = fr.tile([128, QT], f32, tag="neg_mx")
            nc.vector.tensor_scalar_mul(neg_mx[:], mx[:], -1.0)
            e_sb = fr.tile([128, QT * NSEG], f32, tag="e")
            for j in range(QT):
                nc.scalar.activation(e_sb[:, NSEG * j:NSEG * (j + 1)],
                                     segavg_sb[:, NSEG * j:NSEG * (j + 1)],
                                     Exp, bias=neg_mx[:, j:j + 1])
            w_sb = fr.tile([128, QT * NSEG], f32, tag="w")
            nc.vector.tensor_mul(w_sb[:], e_sb[:], cntb[:])
            dsum = fr.tile([128, QT], f32, tag="dsum")
            nc.vector.reduce_sum(dsum[:], w_sb[:].rearrange("p (j n) -> p j n", n=NSEG),
                                 axis=mybir.AxisListType.X)
            rd = fr.tile([128, QT], f32, tag="rd")
            nc.vector.reciprocal(rd[:], dsum[:])
            enorm_sb = fr.tile([128, QT * NSEG], f32, tag="enorm")
            for j in range(QT):
                nc.vector.tensor_scalar_mul(enorm_sb[:, NSEG * j:NSEG * (j + 1)],
                                            e_sb[:, NSEG * j:NSEG * (j + 1)],
                                            rd[:, j:j + 1])
            st["enorm"] = enorm_sb

        def emit_enT(b):
            # EnormT [n, q] fp16 + a row of ones (row 64) for the fused bias
            st = state[b]
            pe = ps_a.tile([NSEG, Q], f32, tag="a")
            for j in range(QT):
                nc.tensor.transpose(pe[0:NSEG, 128 * j:128 * (j + 1)],
                                    st["enorm"][:, NSEG * j:NSEG * (j + 1)], ident[:])
            enT_sb = mid.tile([NSEG + 1, Q], f16, tag="enT")
            nc.vector.tensor_copy(enT_sb[0:NSEG, :], pe[:])
            nc.vector.memset(enT_sb[NSEG:NSEG + 1, :], 1.0)
            st["enT"] = enT_sb

        def emit_cw(b):
            # CW[n, c] = (Csum @ W1^T)[n, c]; row 64 = bias (for the K=65 MM)
            st = state[b]
            csumT_sb = st["csumT"]
            cw = []
            for eb in range(2):
                pcw = ps_a.tile([NSEG, 512], f32, tag="a")
                for d in range(DT):
                    nc.tensor.matmul(pcw[:], csumT_sb[:, NSEG * d:NSEG * (d + 1)],
                                     wtlo_sb[:, _wt_col(eb, d):_wt_col(eb, d) + 512],
                                     start=(d == 0), stop=(d == DT - 1))
                cw_sb = mid.tile([NSEG + 1, 512], f16, tag=f"cw{eb}")
                nc.vector.tensor_copy(cw_sb[0:NSEG, :], pcw[:])
                nc.vector.tensor_copy(cw_sb[NSEG:NSEG + 1, :],
                                      biasr_sb[0:1, 512 * eb:512 * (eb + 1)])
                cw.append(cw_sb)
            st["cw"] = cw

        def emit_align(b, out_eng):
            # align[q, s] = Enorm[q, seg[s]] via one-hot^T broadcast matmul
            st = state[b]
            enT_sb, ohT = st["enT"], st["ohT"]
            for j in range(QT):
                stg = stage.tile([128, S], f16, tag="al_st")
                for h in range(2):
                    pa = ps_al.tile([128, 512], f32, tag="al")
                    nc.tensor.matmul(pa[:], enT_sb[0:NSEG, 128 * j:128 * (j + 1)],
                                     ohT[:, 512 * h:512 * (h + 1)],
                                     start=True, stop=True)
                    nc.vector.tensor_copy(stg[:, 512 * h:512 * (h + 1)], pa[:])
                out_eng.dma_start(align_o.ap()[b, 128 * j:128 * (j + 1), :], stg[:])

        def emit_m3(b, out_eng):
            # out = tanh(O @ W2^T + Enorm @ CW)   (CW already carries the bias)
            st = state[b]
            ot_sb, enT_sb, cw = st["ot"], st["enT"], st["cw"]
            for j in range(QT):
                ost = stage.tile([128, D], f16, tag="out_st")
                for eb in range(2):
                    po = ps_o.tile([128, 512], f32, tag="po")
                    for f in range(DT):
                        nc.tensor.matmul(po[:],
                                         ot_sb[:, f * Q + 128 * j:f * Q + 128 * (j + 1)],
                                         wthi_sb[:, _wt_col(eb, DT + f):_wt_col(eb, DT + f) + 512],
                                         start=(f == 0), stop=False)
                    nc.tensor.matmul(po[:], enT_sb[:, 128 * j:128 * (j + 1)],
                                     cw[eb][:], start=False, stop=True)
                    nc.scalar.activation(ost[:, 512 * eb:512 * (eb + 1)], po[:], Tanh)
                out_eng.dma_start(out_o.ap()[b, 128 * j:128 * (j + 1), :], ost[:])

        # ---- DMA issue plan ----
        # ALL inputs serialized on ONE HWDGE queue (scalar — its engine starts
        # ~0.25us into the kernel vs ~3.8us for sync) in exact consumption
        # order: per-engine FIFO gives every transfer the full HBM bandwidth
        # instead of a fair-share crawl across queues.  Tiny aux tensors ride
        # gpsimd (SWDGE) concurrently; outputs ride sync (idle until ~20us).
        emit_loads_c(0, nc.scalar)
        nc.scalar.dma_start(ident[:], ident_in.ap())
        nc.scalar.dma_start(identr[:], identr_in.ap())
        emit_loads_ot(0, nc.scalar)
        emit_loads_c(1, nc.scalar)
        emit_loads_ot(1, nc.scalar)
        nc.scalar.dma_start(wtlo_sb[:], wtlo_in.ap())
        nc.scalar.dma_start(wthi_sb[:], wthi_in.ap())
        emit_loads_aux(0, nc.gpsimd)
        emit_loads_aux(1, nc.gpsimd)

        # ---- compute schedule (PE kept dense; 2 batches software-pipelined) ----
        emit_warmup(56, 8)
        emit_csum(0)
        emit_csumT(0)
        emit_segavgT(0)
        emit_segavg(0)
        emit_softmax(0)
        emit_csum(1)
        emit_csumT(1)
        emit_segavgT(1)
        emit_segavg(1)
        emit_softmax(1)
        emit_enT(0)
        emit_align(0, nc.sync)
        emit_cw(0)
        emit_cw(1)
        emit_enT(1)
        emit_align(1, nc.sync)
        emit_m3(0, nc.sync)
        emit_m3(1, nc.sync)

    nc.compile()
    return nc


def _host_prep(output, context, W_weight, W_bias, segment_ids):
    """Shard over batch + build per-core input maps (host-side packing)."""
    wt_full = W_weight.T.astype(np.float16)                            # [2D, D]
    wtlo = np.empty((128, 16 * 512), dtype=np.float16)
    wthi = np.empty((128, 16 * 512), dtype=np.float16)
    for eb in range(2):
        for f in range(16):
            dst = wtlo if f < DT else wthi
            col = _wt_col(eb, f)
            dst[:, col:col + 512] = wt_full[128 * f:128 * (f + 1),
                                            512 * eb:512 * (eb + 1)]
    biasr = np.ascontiguousarray(W_bias.astype(np.float16)[None, :])
    ident = np.eye(128, dtype=np.float32)

    in_maps = []
    for c in range(NCORES):
        lo = c * BPC
        cs, ots, ohps, ohTs, cntbs, invcs = [], [], [], [], [], []
        for b in range(BPC):
            ctx = context[lo + b].astype(np.float16)                  # [S, D]
            cs.append(ctx.reshape(ST, 128, D).transpose(1, 0, 2).reshape(128, ST * D))
            otb = output[lo + b].T.astype(np.float16)                 # [D, Q]
            ots.append(otb.reshape(DT, 128, Q).transpose(1, 0, 2).reshape(128, DT * Q))
            ids = segment_ids[lo + b].astype(np.int32)                # [S]
            oh = (ids[:, None] == np.arange(NSEG, dtype=np.int32)[None, :])
            ohf = oh.astype(np.float16)                               # [S, NSEG]
            cnt = oh.sum(axis=0).astype(np.float32)                   # [NSEG]
            ohps.append(np.ascontiguousarray(
                ohf.reshape(ST, 128, NSEG).transpose(1, 0, 2).reshape(128, ST * NSEG)))
            ohTs.append(np.ascontiguousarray(ohf.T))                  # [NSEG, S]
            cntbs.append(np.ascontiguousarray(
                np.broadcast_to(np.tile(cnt, QT)[None, :], (128, QT * NSEG))))
            invcs.append((1.0 / np.maximum(cnt, 1.0)).astype(np.float32)[:, None])
        in_maps.append({
            "ident_in": ident,
            "identr_in": ident,
            "c_in": np.ascontiguousarray(np.stack(cs)),
            "ot_in": np.ascontiguousarray(np.stack(ots)),
            "wtlo_in": wtlo,
            "wthi_in": wthi,
            "ohp_in": np.stack(ohps),
            "ohT_in": np.stack(ohTs),
            "cntb_in": np.stack(cntbs),
            "invc_in": np.stack(invcs),
            "biasr_in": biasr,
        })
    return in_maps


def _run(inputs, trace=False, tmpdir=None):
    from concourse.bass_utils import run_bass_kernel_spmd
    if "nc" not in _CACHE:
        _CACHE["nc"] = _build_nc()
    nc = _CACHE["nc"]
    in_maps = _host_prep(**inputs)
    kw = {}
    if trace:
        kw = {"trace": True, "tmpdir": tmpdir}
    res = run_bass_kernel_spmd(nc, in_maps, core_ids=list(range(NCORES)), **kw)
    out = np.concatenate([res.results[c]["out_o"] for c in range(NCORES)],
                         axis=0).astype(np.float32)
    align = np.concatenate([res.results[c]["align_o"] for c in range(NCORES)],
                           axis=0).astype(np.float32)
    return (out, align), res


def kernel(output, context, W_weight, W_bias, segment_ids):
    # Force host numpy up front: if the caller hands us jax arrays, numpy
    # ops would otherwise dispatch to the accelerator backend.
    (out, align), _ = _run(dict(
        output=np.asarray(output, dtype=np.float32),
        context=np.asarray(context, dtype=np.float32),
        W_weight=np.asarray(W_weight, dtype=np.float32),
        W_bias=np.asarray(W_bias, dtype=np.float32),
        segment_ids=np.asarray(segment_ids, dtype=np.int32)))
    return out, align


# revision 16
# speedup vs baseline: 1.7407x; 1.0148x over previous
"""Trainium2 Bass kernel for nn_Attn_30820685316537 (segment_reduce attention).

Reference computation (per batch b):
    score = output @ context^T                     [Q, S]
    avg   = per-segment mean of score over S, broadcast back
    align = softmax(avg, axis=S)                   [Q, S]
    ac    = align @ context                        [Q, D]
    out   = tanh(concat(ac, output) @ W^T + bias)  [Q, D]
    returns (out, align)

Algebraic structure exploited on device: `avg` is constant within each of the
64 contiguous segments, so the whole pipeline factors through rank-64 segment
space.  With Csum[n, d] = sum_{s in segment n} context[s, d]:
    segavg[q, n]  = (output[q, :] . Csum[n, :]) / max(cnt[n], 1)
    Enorm[q, n]   = softmax weights per segment (exp/sum with exact counts)
    align[q, s]   = Enorm[q, seg[s]]        (broadcast via 0/1 one-hot matmul)
and additionally the final projection's aligned-context half also factors:
    ac @ W1^T     = Enorm @ (Csum @ W1^T)   (CW := Csum @ W1^T is [64, D])
so aligned_context is never materialised; the M3 GEMM shrinks from
[Q,2D]@[2D,D] to [Q,D]@[D,D] (the output half) plus two rank-64 products.

Sharding: data-parallel over batch B=16 across 8 NeuronCores (2 batches per
core); W replicated.  All heavy streams are fp16 on the DMA path (PE rate is
dtype-independent; DMA bytes halve); the softmax middle section stays fp32.
A burst of warm-up matmuls at kernel start flips the PE HAM clock-gate to
full rate while the first context DMA is still in flight, and the emission
order keeps the PE dense (2 batches software-pipelined) so it stays warm.
"""
import numpy as np
from contextlib import ExitStack

B, Q, S, D = 16, 512, 1024, 1024
NSEG = 64
NCORES = 8
BPC = B // NCORES          # batches per core
QT = Q // 128              # 4 q-tiles
ST = S // 128              # 8 s-tiles
DT = D // 128              # 8 d-tiles

_CACHE = {}


def _wt_col(eb, f):
    """Column offset of W^T tile (eb, f) inside its packed half (lo: f<8 —
    the CW half; hi: f>=8 — the output half)."""
    return (eb * DT + f % DT) * 512


def _build_nc():
    import concourse.bacc as bacc
    import concourse.tile as tile
    import concourse.mybir as mybir

    f32 = mybir.dt.float32
    f32r = mybir.dt.float32r
    f16 = mybir.dt.float16

    nc = bacc.Bacc("TRN2", target_bir_lowering=False, debug=False,
                   enable_asserts=False, num_devices=NCORES)

    ident_in = nc.dram_tensor("ident_in", [128, 128], f32, kind="ExternalInput")
    identr_in = nc.dram_tensor("identr_in", [128, 128], f32r, kind="ExternalInput")
    c_in = nc.dram_tensor("c_in", [BPC, 128, ST * D], f16, kind="ExternalInput")
    ot_in = nc.dram_tensor("ot_in", [BPC, 128, DT * Q], f16, kind="ExternalInput")
    wtlo_in = nc.dram_tensor("wtlo_in", [128, 16 * 512], f16, kind="ExternalInput")
    wthi_in = nc.dram_tensor("wthi_in", [128, 16 * 512], f16, kind="ExternalInput")
    ohp_in = nc.dram_tensor("ohp_in", [BPC, 128, ST * NSEG], f16, kind="ExternalInput")
    ohT_in = nc.dram_tensor("ohT_in", [BPC, NSEG, S], f16, kind="ExternalInput")
    cntb_in = nc.dram_tensor("cntb_in", [BPC, 128, QT * NSEG], f32, kind="ExternalInput")
    invc_in = nc.dram_tensor("invc_in", [BPC, NSEG, 1], f32, kind="ExternalInput")
    biasr_in = nc.dram_tensor("biasr_in", [1, D], f16, kind="ExternalInput")

    out_o = nc.dram_tensor("out_o", [BPC, Q, D], f16, kind="ExternalOutput")
    align_o = nc.dram_tensor("align_o", [BPC, Q, S], f16, kind="ExternalOutput")

    Exp = mybir.ActivationFunctionType.Exp
    Tanh = mybir.ActivationFunctionType.Tanh

    with tile.TileContext(nc) as tc, ExitStack() as ctx:
        consts = ctx.enter_context(tc.tile_pool(name="consts", bufs=1))
        wt_pool = ctx.enter_context(tc.tile_pool(name="wt", bufs=1))
        c_pool = ctx.enter_context(tc.tile_pool(name="cp", bufs=2))
        ot_pool = ctx.enter_context(tc.tile_pool(name="otp", bufs=2))
        aux = ctx.enter_context(tc.tile_pool(name="aux", bufs=2))
        mid = ctx.enter_context(tc.tile_pool(name="mid", bufs=2))
        fr = ctx.enter_context(tc.tile_pool(name="fr", bufs=2))
        stage = ctx.enter_context(tc.tile_pool(name="stage", bufs=3))

        ps_a = ctx.enter_context(tc.tile_pool(name="ps_a", bufs=3, space="PSUM"))
        ps_al = ctx.enter_context(tc.tile_pool(name="ps_al", bufs=2, space="PSUM"))
        ps_o = ctx.enter_context(tc.tile_pool(name="ps_o", bufs=3, space="PSUM"))

        # ---- constants / weights ----
        ident = consts.tile([128, 128], f32, tag="ident")
        identr = consts.tile([128, 128], f32r, tag="identr")
        biasr_sb = consts.tile([1, D], f16, tag="biasr")
        nc.gpsimd.dma_start(biasr_sb[:], biasr_in.ap())
        wtlo_sb = wt_pool.tile([128, 16 * 512], f16, tag="wtlo")
        wthi_sb = wt_pool.tile([128, 16 * 512], f16, tag="wthi")
        junk = consts.tile([128, 128], f16, tag="junk")

        state = [dict() for _ in range(BPC)]

        def emit_loads_aux(b, eng):
            st = state[b]
            ohp = aux.tile([128, ST * NSEG], f16, tag="ohp")
            eng.dma_start(ohp[:], ohp_in.ap()[b])
            ohT = aux.tile([NSEG, S], f16, tag="ohT")
            eng.dma_start(ohT[:], ohT_in.ap()[b])
            cntb = aux.tile([128, QT * NSEG], f32, tag="cntb")
            eng.dma_start(cntb[:], cntb_in.ap()[b])
            invc = aux.tile([NSEG, 1], f32, tag="invc")
            eng.dma_start(invc[:], invc_in.ap()[b])
            st["ohp"], st["ohT"], st["cntb"], st["invc"] = ohp, ohT, cntb, invc

        def emit_loads_c(b, eng):
            # two half-DMAs so Csum can start on the first half early
            ca = c_pool.tile([128, ST * D // 2], f16, tag="ca")
            eng.dma_start(ca[:], c_in.ap()[b][:, 0:ST * D // 2])
            cb = c_pool.tile([128, ST * D // 2], f16, tag="cb")
            eng.dma_start(cb[:], c_in.ap()[b][:, ST * D // 2:ST * D])
            state[b]["c"] = (ca, cb)

        def emit_loads_ot(b, eng):
            ot_sb = ot_pool.tile([128, DT * Q], f16, tag="ot")
            eng.dma_start(ot_sb[:], ot_in.ap()[b])
            state[b]["ot"] = ot_sb

        def emit_warmup(n_small, n_big):
            # Real matmuls on a memset tile (no DMA dependency; transposes
            # don't count as PE-busy for the HAM clock gate): flip the PE to
            # 2.4 GHz and keep it busy while the first input DMAs stream.
            nc.vector.memset(junk[:], 1.0)
            wps = ps_o.tile([128, 512], f32, tag="po")
            for _ in range(n_small):
                nc.tensor.matmul(wps[:, 0:128], junk[:], junk[:],
                                 start=True, stop=True)
            for _ in range(n_big):
                for h in range(4):
                    nc.tensor.matmul(wps[:, 128 * h:128 * (h + 1)],
                                     junk[:], junk[:], start=True, stop=True)

        def emit_csum(b):
            # Csum[n, d] = sum_{s in seg n} C[s, d]
            st = state[b]
            ohp = st["ohp"]
            cs0 = ps_a.tile([NSEG, 512], f32, tag="a")
            cs1 = ps_a.tile([NSEG, 512], f32, tag="a")
            for i in range(ST):
                c_half = st["c"][i // 4]
                ii = i % 4
                nc.tensor.matmul(cs0[:], ohp[:, NSEG * i:NSEG * (i + 1)],
                                 c_half[:, ii * D:ii * D + 512],
                                 start=(i == 0), stop=(i == ST - 1))
                nc.tensor.matmul(cs1[:], ohp[:, NSEG * i:NSEG * (i + 1)],
                                 c_half[:, ii * D + 512:ii * D + 1024],
                                 start=(i == 0), stop=(i == ST - 1))
            csum_sb = fr.tile([NSEG, D], f32r, tag="csum")
            nc.vector.tensor_copy(csum_sb[:, 0:512], cs0[:])
            nc.vector.tensor_copy(csum_sb[:, 512:1024], cs1[:])
            st["csum"] = csum_sb

        def emit_csumT(b):
            # csumT [128, (d n)] fp16 via 8 PE transposes of [64, 128] blocks
            st = state[b]
            csum_sb = st["csum"]
            pt = ps_a.tile([128, NSEG * DT], f32r, tag="a")
            for d in range(DT):
                nc.tensor.transpose(pt[:, NSEG * d:NSEG * (d + 1)],
                                    csum_sb[0:NSEG, 128 * d:128 * (d + 1)],
                                    identr[0:NSEG, 0:NSEG])
            csumT_sb = mid.tile([128, NSEG * DT], f16, tag="csumT")
            nc.vector.tensor_copy(csumT_sb[:], pt[:])
            st["csumT"] = csumT_sb

        def emit_segavgT(b):
            # segavgT[n, q] = (Csum @ O^T)[n, q] * invc[n]
            st = state[b]
            csumT_sb, ot_sb, invc = st["csumT"], st["ot"], st["invc"]
            sg = ps_a.tile([NSEG, Q], f32, tag="a")
            for d in range(DT):
                nc.tensor.matmul(sg[:], csumT_sb[:, NSEG * d:NSEG * (d + 1)],
                                 ot_sb[:, d * Q:(d + 1) * Q],
                                 start=(d == 0), stop=(d == DT - 1))
            segavgT_sb = fr.tile([NSEG, Q], f32, tag="segavgT")
            nc.vector.tensor_scalar_mul(segavgT_sb[:], sg[:], invc[:])
            st["segavgT"] = segavgT_sb

        def emit_segavg(b):
            # segavg [q, (j n)] via 4 exact fp32 transposes
            st = state[b]
            pt = ps_a.tile([128, QT * NSEG], f32, tag="a")
            for j in range(QT):
                nc.tensor.transpose(pt[:, NSEG * j:NSEG * (j + 1)],
                                    st["segavgT"][0:NSEG, 128 * j:128 * (j + 1)],
                                    ident[0:NSEG, 0:NSEG])
            segavg_sb = fr.tile([128, QT * NSEG], f32, tag="segavg")
            nc.vector.tensor_copy(segavg_sb[:], pt[:])
            st["segavg"] = segavg_sb

        def emit_softmax(b):
            # softmax over segments with exact counts (all fp32)
            st = state[b]
            segavg_sb, cntb = st["segavg"], st["cntb"]
            mx = fr.tile([128, QT], f32, tag="mx")
            nc.vector.reduce_max(mx[:], segavg_sb[:].rearrange("p (j n) -> p j n", n=NSEG),
                                 axis=mybir.AxisListType.X)
            neg_mx = fr.tile([128, QT], f32, tag="neg_mx")
            nc.vector.tensor_scalar_mul(neg_mx[:], mx[:], -1.0)
            e_sb = fr.tile([128, QT * NSEG], f32, tag="e")
            for j in range(QT):
                nc.scalar.activation(e_sb[:, NSEG * j:NSEG * (j + 1)],
                                     segavg_sb[:, NSEG * j:NSEG * (j + 1)],
                                     Exp, bias=neg_mx[:, j:j + 1])
            w_sb = fr.tile([128, QT * NSEG], f32, tag="w")
            nc.vector.tensor_mul(w_sb[:], e_sb[:], cntb[:])
            dsum = fr.tile([128, QT], f32, tag="dsum")
            nc.vector.reduce_sum(dsum[:], w_sb[:].rearrange("p (j n) -> p j n", n=NSEG),
                                 axis=mybir.AxisListType.X)
            rd = fr.tile([128, QT], f32, tag="rd")
            nc.vector.reciprocal(rd[:], dsum[:])
            enorm_sb = fr.tile([128, QT * NSEG], f32, tag="enorm")
            for j in range(QT):
                nc.vector.tensor_scalar_mul(enorm_sb[:, NSEG * j:NSEG * (j + 1)],
                                            e_sb[:, NSEG * j:NSEG * (j + 1)],
                                            rd[:, j:j + 1])
            st["enorm"] = enorm_sb

        def emit_enT(b):
            # EnormT [n, q] fp16 + a row of ones (row 64) for the fused bias
            st = state[b]
            pe = ps_a.tile([NSEG, Q], f32, tag="a")
            for j in range(QT):
                nc.tensor.transpose(pe[0:NSEG, 128 * j:128 * (j + 1)],
                                    st["enorm"][:, NSEG * j:NSEG * (j + 1)], ident[:])
            enT_sb = mid.tile([NSEG + 1, Q], f16, tag="enT")
            nc.vector.tensor_copy(enT_sb[0:NSEG, :], pe[:])
            nc.vector.memset(enT_sb[NSEG:NSEG + 1, :], 1.0)
            st["enT"] = enT_sb

        def emit_cw_packed():
            # CW[n, c] = (Csum @ W1^T)[n, c] for BOTH batches concurrently:
            # M=64 outputs column-tiled into partition halves of one PSUM bank
            # (tile_position derives from out.base_partition), so each b0/b1
            # matmul pair runs in the same PE pass.  Row 64 = bias (K=65 MM).
            for eb in range(2):
                pcw = ps_a.tile([128, 512], f32, tag="a")
                for d in range(DT):
                    for b in range(BPC):
                        nc.tensor.matmul(pcw[64 * b:64 * (b + 1), :],
                                         state[b]["csumT"][:, NSEG * d:NSEG * (d + 1)],
                                         wtlo_sb[:, _wt_col(eb, d):_wt_col(eb, d) + 512],
                                         start=(d == 0), stop=(d == DT - 1))
                for b in range(BPC):
                    cw_sb = mid.tile([NSEG + 1, 512], f16, tag=f"cw{eb}")
                    nc.vector.tensor_copy(cw_sb[0:NSEG, :], pcw[64 * b:64 * (b + 1), :])
                    nc.vector.tensor_copy(cw_sb[NSEG:NSEG + 1, :],
                                          biasr_sb[0:1, 512 * eb:512 * (eb + 1)])
                    state[b].setdefault("cw", []).append(cw_sb)

        def emit_align(b, out_eng):
            # align[q, s] = Enorm[q, seg[s]] via one-hot^T broadcast matmul
            st = state[b]
            enT_sb, ohT = st["enT"], st["ohT"]
            for j in range(QT):
                stg = stage.tile([128, S], f16, tag="al_st")
                for h in range(2):
                    pa = ps_al.tile([128, 512], f32, tag="al")
                    nc.tensor.matmul(pa[:], enT_sb[0:NSEG, 128 * j:128 * (j + 1)],
                                     ohT[:, 512 * h:512 * (h + 1)],
                                     start=True, stop=True)
                    nc.vector.tensor_copy(stg[:, 512 * h:512 * (h + 1)], pa[:])
                out_eng.dma_start(align_o.ap()[b, 128 * j:128 * (j + 1), :], stg[:])

        def emit_m3(b, out_eng):
            # out = tanh(O @ W2^T + Enorm @ CW)   (CW already carries the bias)
            st = state[b]
            ot_sb, enT_sb, cw = st["ot"], st["enT"], st["cw"]
            for j in range(QT):
                ost = stage.tile([128, D], f16, tag="out_st")
                for eb in range(2):
                    po = ps_o.tile([128, 512], f32, tag="po")
                    for f in range(DT):
                        nc.tensor.matmul(po[:],
                                         ot_sb[:, f * Q + 128 * j:f * Q + 128 * (j + 1)],
                                         wthi_sb[:, _wt_col(eb, DT + f):_wt_col(eb, DT + f) + 512],
                                         start=(f == 0), stop=False)
                    nc.tensor.matmul(po[:], enT_sb[:, 128 * j:128 * (j + 1)],
                                     cw[eb][:], start=False, stop=True)
                    nc.scalar.activation(ost[:, 512 * eb:512 * (eb + 1)], po[:], Tanh)
                out_eng.dma_start(out_o.ap()[b, 128 * j:128 * (j + 1), :], ost[:])

        # ---- DMA issue plan ----
        # ALL inputs serialized on ONE HWDGE queue (scalar — its engine starts
        # ~0.25us into the kernel vs ~3.8us for sync) in exact consumption
        # order: per-engine FIFO gives every transfer the full HBM bandwidth
        # instead of a fair-share crawl across queues.  Tiny aux tensors ride
        # gpsimd (SWDGE) concurrently; outputs ride sync (idle until ~20us).
        emit_loads_c(0, nc.scalar)
        nc.scalar.dma_start(ident[:], ident_in.ap())
        nc.scalar.dma_start(identr[:], identr_in.ap())
        emit_loads_ot(0, nc.scalar)
        emit_loads_c(1, nc.scalar)
        emit_loads_ot(1, nc.scalar)
        nc.scalar.dma_start(wtlo_sb[:], wtlo_in.ap())
        nc.scalar.dma_start(wthi_sb[:], wthi_in.ap())
        emit_loads_aux(0, nc.gpsimd)
        emit_loads_aux(1, nc.gpsimd)

        # ---- compute schedule (PE kept dense; 2 batches software-pipelined) ----
        emit_warmup(44, 0)
        emit_csum(0)
        emit_csumT(0)
        emit_segavgT(0)
        emit_segavg(0)
        emit_softmax(0)
        emit_csum(1)
        emit_csumT(1)
        emit_segavgT(1)
        emit_segavg(1)
        emit_softmax(1)
        emit_enT(0)
        emit_align(0, nc.sync)
        emit_enT(1)
        emit_align(1, nc.sync)
        emit_cw_packed()
        emit_m3(0, nc.sync)
        emit_m3(1, nc.sync)

    nc.compile()
    return nc


def _host_prep(output, context, W_weight, W_bias, segment_ids):
    """Shard over batch + build per-core input maps (host-side packing)."""
    wt_full = W_weight.T.astype(np.float16)                            # [2D, D]
    wtlo = np.empty((128, 16 * 512), dtype=np.float16)
    wthi = np.empty((128, 16 * 512), dtype=np.float16)
    for eb in range(2):
        for f in range(16):
            dst = wtlo if f < DT else wthi
            col = _wt_col(eb, f)
            dst[:, col:col + 512] = wt_full[128 * f:128 * (f + 1),
                                            512 * eb:512 * (eb + 1)]
    biasr = np.ascontiguousarray(W_bias.astype(np.float16)[None, :])
    ident = np.eye(128, dtype=np.float32)

    in_maps = []
    for c in range(NCORES):
        lo = c * BPC
        cs, ots, ohps, ohTs, cntbs, invcs = [], [], [], [], [], []
        for b in range(BPC):
            ctx = context[lo + b].astype(np.float16)                  # [S, D]
            cs.append(ctx.reshape(ST, 128, D).transpose(1, 0, 2).reshape(128, ST * D))
            otb = output[lo + b].T.astype(np.float16)                 # [D, Q]
            ots.append(otb.reshape(DT, 128, Q).transpose(1, 0, 2).reshape(128, DT * Q))
            ids = segment_ids[lo + b].astype(np.int32)                # [S]
            oh = (ids[:, None] == np.arange(NSEG, dtype=np.int32)[None, :])
            ohf = oh.astype(np.float16)                               # [S, NSEG]
            cnt = oh.sum(axis=0).astype(np.float32)                   # [NSEG]
            ohps.append(np.ascontiguousarray(
                ohf.reshape(ST, 128, NSEG).transpose(1, 0, 2).reshape(128, ST * NSEG)))
            ohTs.append(np.ascontiguousarray(ohf.T))                  # [NSEG, S]
            cntbs.append(np.ascontiguousarray(
                np.broadcast_to(np.tile(cnt, QT)[None, :], (128, QT * NSEG))))
            invcs.append((1.0 / np.maximum(cnt, 1.0)).astype(np.float32)[:, None])
        in_maps.append({
            "ident_in": ident,
            "identr_in": ident,
            "c_in": np.ascontiguousarray(np.stack(cs)),
            "ot_in": np.ascontiguousarray(np.stack(ots)),
            "wtlo_in": wtlo,
            "wthi_in": wthi,
            "ohp_in": np.stack(ohps),
            "ohT_in": np.stack(ohTs),
            "cntb_in": np.stack(cntbs),
            "invc_in": np.stack(invcs),
            "biasr_in": biasr,
        })
    return in_maps


def _run(inputs, trace=False, tmpdir=None):
    from concourse.bass_utils import run_bass_kernel_spmd
    if "nc" not in _CACHE:
        _CACHE["nc"] = _build_nc()
    nc = _CACHE["nc"]
    in_maps = _host_prep(**inputs)
    kw = {}
    if trace:
        kw = {"trace": True, "tmpdir": tmpdir}
    res = run_bass_kernel_spmd(nc, in_maps, core_ids=list(range(NCORES)), **kw)
    out = np.concatenate([res.results[c]["out_o"] for c in range(NCORES)],
                         axis=0).astype(np.float32)
    align = np.concatenate([res.results[c]["align_o"] for c in range(NCORES)],
                           axis=0).astype(np.float32)
    return (out, align), res


def kernel(output, context, W_weight, W_bias, segment_ids):
    # Force host numpy up front: if the caller hands us jax arrays, numpy
    # ops would otherwise dispatch to the accelerator backend.
    (out, align), _ = _run(dict(
        output=np.asarray(output, dtype=np.float32),
        context=np.asarray(context, dtype=np.float32),
        W_weight=np.asarray(W_weight, dtype=np.float32),
        W_bias=np.asarray(W_bias, dtype=np.float32),
        segment_ids=np.asarray(segment_ids, dtype=np.int32)))
    return out, align
